# revision 28
# baseline (speedup 1.0000x reference)
"""DynaLoRALinear Trainium2 kernel (v3).

Data-parallel over batch B across 8 NeuronCores (one sample per core).
Per core:
  - router: logits = pooled @ C.T with C = W_r @ gating_W folded on the
    host (weight-only reassociation), so every core computes the full
    [NMOD, B] logits locally -- no collective at all.
  - gate weights from expert_scores ranks + module_prob>0.5 branch select.
  - base:   out = x_b @ W_base.T + b_base
  - lora:   tT = A_cat @ x_b.T, then out += tT.T @ (B_cat * gate)

All matmul operands are bf16. x_b^T is SBUF-resident (16 MB, 32 separate
k-tiles so compute can pace the incoming DMA stream), W_base^T streams
once through a ring pool (pre-tiled in DRAM, contiguous 128 KB tiles).
Phase A runs chunk 0 k-outer across 8 PSUM banks so the PE stays busy
while x streams in; chunk 0's LoRA term is applied later via an SWDGE
read-modify-write pass once the gate is known.
"""

import sys
import types

import numpy as np

B, L, D, E, R, NMOD = 8, 2048, 4096, 4, 8, 7
N_CORES = 8
ER = E * R          # 32
KT = D // 128       # 32 k-tiles
MT = L // 128       # 16 m-tiles
NCH = D // 512      # 8 output-column chunks
MG = L // 512       # 4 m-groups for the lora-t pass


def _install_profile_hook():
    """Make bass_utils' trace path importable (no-op if already present)."""
    try:
        import antenv.axon_hooks  # noqa: F401
        return
    except ImportError:
        pass
    try:
        import antenv
    except ImportError:
        return
    mod = types.ModuleType("antenv.axon_hooks")
    mod._hook = None
    mod.set_axon_ntff_profile_hook = lambda h: setattr(mod, "_hook", h)
    mod.get_axon_ntff_profile_hook = lambda: mod._hook
    sys.modules["antenv.axon_hooks"] = mod
    antenv.axon_hooks = mod
    try:
        from trn_agent_boot.trn_boot import _ntff_profile_via_ctypes
        hook = _ntff_profile_via_ctypes("/opt/axon/libaxon_pjrt.so")
        if hook is not None:
            mod.set_axon_ntff_profile_hook(hook)
    except Exception:
        pass


_PROGRAM_CACHE = {}


def _build_program(k: int, module_idx: int, has_bias: bool):
    import concourse.mybir as mybir
    import concourse.tile as tile
    from concourse import bacc
    from concourse.masks import make_identity

    f32 = mybir.dt.float32
    bf16 = mybir.dt.bfloat16
    alu = mybir.AluOpType
    act_fn = mybir.ActivationFunctionType

    k_lo = max(1, k // 2)
    w_bufs = 36 if has_bias else 53

    nc = bacc.Bacc("TRN2", target_bir_lowering=False, debug=False,
                   num_devices=N_CORES)

    # --- DRAM I/O -------------------------------------------------------
    xT = nc.dram_tensor("xT", [D, L], bf16, kind="ExternalInput")
    Wt = nc.dram_tensor("Wt", [NCH, KT, 128, 512], bf16,
                        kind="ExternalInput")
    ctT = nc.dram_tensor("ctT", [D, NMOD], bf16, kind="ExternalInput")
    pooledT = nc.dram_tensor("pooledT", [D, B], bf16, kind="ExternalInput")
    scores_f = nc.dram_tensor("scores_f", [1, E * B], f32,
                              kind="ExternalInput")
    A_rhs = nc.dram_tensor("A_rhs", [D, ER], bf16, kind="ExternalInput")
    B_cat = nc.dram_tensor("B_cat", [ER, D], bf16, kind="ExternalInput")
    b_row = nc.dram_tensor("b_row", [1, D], f32, kind="ExternalInput")
    msel = nc.dram_tensor("msel", [ER, E * B], f32, kind="ExternalInput")
    out = nc.dram_tensor("out", [L, D], f32, kind="ExternalOutput")

    with tile.TileContext(nc) as tc:
        with (
            tc.tile_pool(name="const", bufs=1) as const_pool,
            tc.tile_pool(name="gatep", bufs=1) as gate_pool,
            tc.tile_pool(name="rsb", bufs=1) as rsb,
            tc.tile_pool(name="xsb", bufs=KT) as xsb_pool,
            tc.tile_pool(name="wpool", bufs=w_bufs) as wpool,
            tc.tile_pool(name="apool", bufs=1) as apool,
            tc.tile_pool(name="tpool", bufs=1) as tpool,
            tc.tile_pool(name="ballp", bufs=1) as ball_pool,
            tc.tile_pool(name="epool", bufs=4) as epool,
            tc.tile_pool(name="biasp",
                         bufs=(NCH if has_bias else 1)) as biasp,
            tc.tile_pool(name="mps", bufs=8, space="PSUM") as mps,
        ):
            ident = const_pool.tile([128, 128], f32)
            make_identity(nc, ident)
            gate32 = gate_pool.tile([ER, 1], f32)

            # HAM warm-up: dense matmuls on a zeroed tile bring the PE
            # clock gate to 8/8 while the first input DMAs stream.
            # (memset, not ident, so no wait on make_identity)
            idb = const_pool.tile([128, 128], bf16)
            nc.vector.memset(idb[:], 0.0)
            warm = mps.tile([128, 128], f32, tag="ps", name="warm")
            for i in range(64):
                nc.tensor.matmul(warm[:], idb[:], idb[:],
                                 start=(i == 0), stop=(i == 63))

            bias_all = []
            if has_bias:
                for hh in range(NCH):
                    bias_bc = biasp.tile([128, 512], f32, tag="biasbc",
                                         name=f"biasbc_{hh}")
                    nc.sync.dma_start(
                        bias_bc[0:1, :],
                        b_row[:, hh * 512:(hh + 1) * 512])
                    nc.gpsimd.partition_broadcast(bias_bc[:],
                                                  bias_bc[0:1, :])
                    bias_all.append(bias_bc)

            # ====== small input DMAs ===================================
            ct_sb = rsb.tile([128, KT, NMOD], bf16)
            nc.sync.dma_start(
                ct_sb[:], ctT[:].rearrange("(a p) m -> p a m", p=128))
            pt_sb = rsb.tile([128, KT, B], bf16)
            nc.sync.dma_start(
                pt_sb[:], pooledT[:].rearrange("(a p) m -> p a m", p=128))
            msel_sb = rsb.tile([ER, E * B], f32)
            nc.sync.dma_start(msel_sb[:], msel[:])
            sc = rsb.tile([1, E * B], f32)
            nc.sync.dma_start(sc[:], scores_f[:])
            a_sb = apool.tile([128, KT, ER], bf16)
            nc.sync.dma_start(
                a_sb[:], A_rhs[:].rearrange("(a p) m -> p a m", p=128))
            bg0 = rsb.tile([ER, 512], bf16)
            nc.sync.dma_start(bg0[:], B_cat[:, 0:512])

            # ====== bulk DMAs up front: x via two queues + W chunk 0 ====
            x_tiles = []
            wt0 = []
            tT = tpool.tile([ER, L], bf16)
            for kt in range(KT):
                xs = xsb_pool.tile([128, L], bf16, tag="x",
                                   name=f"x_{kt}")
                nc.sync.dma_start(xs[:], xT[kt * 128:(kt + 1) * 128, :])
                x_tiles.append(xs)
                wt = wpool.tile([128, 512], bf16, tag="w",
                                name=f"w_0_{kt}")
                nc.sync.dma_start(wt[:], Wt[0, kt])
                wt0.append(wt)

            # ====== router: logits = pooled @ C.T (local, no collective)
            ps_r = mps.tile([NMOD, B], f32, tag="ps", name="ps_r")
            for kt in range(KT):
                nc.tensor.matmul(ps_r[:], ct_sb[:, kt, :], pt_sb[:, kt, :],
                                 start=(kt == 0), stop=(kt == KT - 1))
            lr_sb = rsb.tile([NMOD, B], f32)
            nc.vector.tensor_copy(lr_sb[:], ps_r[:])

            # collective-independent: expert ranks from scores
            rank = rsb.tile([1, E * B], f32)
            nc.vector.memset(rank[:], 0.0)
            tmp = rsb.tile([1, B], f32)
            for e in range(E):
                re = rank[:, e * B:(e + 1) * B]
                se = sc[:, e * B:(e + 1) * B]
                for e2 in range(E):
                    if e2 == e:
                        continue
                    s2 = sc[:, e2 * B:(e2 + 1) * B]
                    nc.vector.tensor_tensor(tmp[:], s2, se, op=alu.is_gt)
                    nc.vector.tensor_add(re, re, tmp[:])
                    if e2 < e:
                        nc.vector.tensor_tensor(tmp[:], s2, se,
                                                op=alu.is_equal)
                        nc.vector.tensor_add(re, re, tmp[:])
            w_hi = rsb.tile([1, E * B], f32)
            nc.vector.tensor_scalar(w_hi[:], rank[:], float(k),
                                    1.0 / float(k),
                                    op0=alu.is_lt, op1=alu.mult)
            w_lo = rsb.tile([1, E * B], f32)
            nc.vector.tensor_scalar(w_lo[:], rank[:], float(k_lo),
                                    1.0 / float(k_lo),
                                    op0=alu.is_lt, op1=alu.mult)
            diff = rsb.tile([1, E * B], f32)
            nc.vector.tensor_sub(diff[:], w_hi[:], w_lo[:])

            # ====== router part B: softmax branch -> per-core gate ======
            ltp = mps.tile([B, NMOD], f32, tag="ps", name="ltp")
            nc.tensor.transpose(ltp[:], lr_sb[:], ident[0:NMOD, 0:NMOD])
            lt = rsb.tile([B, NMOD], f32)
            nc.vector.tensor_copy(lt[:], ltp[:])
            mx = rsb.tile([B, 1], f32)
            nc.vector.tensor_reduce(out=mx[:], in_=lt[:], op=alu.max,
                                    axis=mybir.AxisListType.X)
            mxn = rsb.tile([B, 1], f32)
            nc.vector.tensor_scalar_mul(mxn[:], mx[:], -1.0)
            ex = rsb.tile([B, NMOD], f32)
            nc.scalar.activation(ex[:], lt[:], act_fn.Exp, bias=mxn[:])
            sm = rsb.tile([B, 1], f32)
            nc.vector.tensor_reduce(out=sm[:], in_=ex[:], op=alu.add,
                                    axis=mybir.AxisListType.X)
            rs = rsb.tile([B, 1], f32)
            nc.vector.reciprocal(rs[:], sm[:])
            p0 = rsb.tile([B, 1], f32)
            nc.vector.tensor_mul(
                p0[:], ex[:, module_idx:module_idx + 1], rs[:])
            hi = rsb.tile([B, 1], f32)
            nc.vector.tensor_single_scalar(hi[:], p0[:], 0.5, alu.is_gt)
            hp = mps.tile([1, B], f32, tag="ps", name="hp")
            nc.tensor.transpose(hp[:], hi[:], ident[0:B, 0:B])
            hi_row = rsb.tile([1, B], f32)
            nc.vector.tensor_copy(hi_row[:], hp[:])
            gate = rsb.tile([1, E * B], f32)
            for e in range(E):
                nc.vector.tensor_mul(gate[:, e * B:(e + 1) * B],
                                     diff[:, e * B:(e + 1) * B],
                                     hi_row[:])
            nc.vector.tensor_add(gate[:], gate[:], w_lo[:])
            gateb = rsb.tile([ER, E * B], f32)
            nc.gpsimd.partition_broadcast(gateb[:], gate[:])
            g32m = rsb.tile([ER, E * B], f32)
            nc.vector.tensor_tensor(g32m[:], gateb[:], msel_sb[:],
                                    op=alu.mult)
            nc.vector.tensor_reduce(out=gate32[:], in_=g32m[:],
                                    op=alu.add,
                                    axis=mybir.AxisListType.X)
            nc.vector.tensor_scalar_mul(bg0[:], bg0[:], gate32[:, 0:1])

            # ====== phase A: chunk-0 mt0..3 + all lora-t, k-outer =======
            # (8 PSUM groups -> 8 matmuls per arriving x k-tile, so the
            # PE paces the x DMA stream instead of stalling behind it)
            psA = [mps.tile([128, 512], f32, tag="ps", name=f"psA_{mt}")
                   for mt in range(4)]
            ps_t = [mps.tile([ER, 512], f32, tag="ps", name=f"pst_{mg}")
                    for mg in range(MG)]
            for kt in range(KT):
                st, sp = (kt == 0), (kt == KT - 1)
                for mg in range(MG):
                    nc.tensor.matmul(
                        ps_t[mg][:], a_sb[:, kt, :],
                        x_tiles[kt][:, mg * 512:(mg + 1) * 512],
                        start=st, stop=sp)
                for mt in range(4):
                    nc.tensor.matmul(psA[mt][:],
                                     x_tiles[kt][:, mt * 128:(mt + 1) * 128],
                                     wt0[kt][:], start=st, stop=False)
            b_all = ball_pool.tile([ER, NCH, 512], bf16)
            nc.sync.dma_start(
                b_all[:], B_cat[:].rearrange("p (c n) -> p c n", c=NCH))
            for mg in range(MG):
                nc.vector.tensor_copy(tT[:, mg * 512:(mg + 1) * 512],
                                      ps_t[mg][:])
            for mt in range(4):
                nc.tensor.matmul(psA[mt][:],
                                 tT[:, mt * 128:(mt + 1) * 128],
                                 bg0[:], start=False, stop=True)
                ev = epool.tile([128, 512], f32, tag="ev",
                                name=f"evA_{mt}")
                if has_bias:
                    nc.vector.tensor_add(ev[:], psA[mt][:],
                                         bias_all[0][:])
                elif mt % 2 == 0:
                    nc.vector.tensor_copy(ev[:], psA[mt][:])
                else:
                    nc.scalar.activation(ev[:], psA[mt][:], act_fn.Copy)
                nc.sync.dma_start(
                    out[mt * 128:(mt + 1) * 128, 0:512], ev[:])

            # gate-scale B in place, one slice per output chunk
            bg_tiles = [bg0[:]]
            for c in range(1, NCH):
                nc.vector.tensor_scalar_mul(b_all[:, c, :],
                                            b_all[:, c, :],
                                            gate32[:, 0:1])
                bg_tiles.append(b_all[:, c, :])

            # W-tile ring: prefetch chunk c+1's tiles while chunk c runs.
            w_next = {}

            def prefetch_w(c, kts):
                if c >= NCH:
                    return
                row = w_next.setdefault(c, [None] * KT)
                for kt in kts:
                    if kt >= KT or row[kt] is not None:
                        continue
                    wt = wpool.tile([128, 512], bf16, tag="w",
                                    name=f"w_{c}_{kt}")
                    nc.sync.dma_start(wt[:], Wt[c, kt])
                    row[kt] = wt

            # ====== phase B: rest of chunk 0 (mt4..15, lora first) ======
            for mt in range(4, MT):
                j = mt - 4
                prefetch_w(1, range(j * 3, j * 3 + 3))
                ps = mps.tile([128, 512], f32, tag="ps", name=f"ps_0_{mt}")
                nc.tensor.matmul(ps[:], tT[:, mt * 128:(mt + 1) * 128],
                                 bg0[:], start=True, stop=False)
                for kt in range(KT):
                    nc.tensor.matmul(
                        ps[:], x_tiles[kt][:, mt * 128:(mt + 1) * 128],
                        wt0[kt][:], start=False, stop=(kt == KT - 1))
                ev = epool.tile([128, 512], f32, tag="ev",
                                name=f"ev_0_{mt}")
                if has_bias:
                    nc.vector.tensor_add(ev[:], ps[:], bias_all[0][:])
                elif mt % 2 == 0:
                    nc.vector.tensor_copy(ev[:], ps[:])
                else:
                    nc.scalar.activation(ev[:], ps[:], act_fn.Copy)
                nc.sync.dma_start(
                    out[mt * 128:(mt + 1) * 128, 0:512], ev[:])

            # ====== phase C: chunks 1..7 (lora MM first, then base) =====
            # chunk 2's mt loop also carries the deferred chunk-0 lora
            # (SWDGE accumulate), one m-tile per iteration.
            def emit_chunk(c):
                prefetch_w(c, range(KT))
                wt_c = w_next[c]
                for mt in range(MT):
                    prefetch_w(c + 1, range(mt * 2, mt * 2 + 2))
                    ps = mps.tile([128, 512], f32, tag="ps",
                                  name=f"ps_{c}_{mt}")
                    nc.tensor.matmul(ps[:], tT[:, mt * 128:(mt + 1) * 128],
                                     bg_tiles[c],
                                     start=True, stop=False)
                    for kt in range(KT):
                        nc.tensor.matmul(
                            ps[:], x_tiles[kt][:, mt * 128:(mt + 1) * 128],
                            wt_c[kt][:], start=False, stop=(kt == KT - 1))
                    ev = epool.tile([128, 512], f32, tag="ev",
                                    name=f"ev_{c}_{mt}")
                    if has_bias:
                        nc.vector.tensor_add(ev[:], ps[:], bias_all[c][:])
                    elif mt % 2 == 0:
                        nc.vector.tensor_copy(ev[:], ps[:])
                    else:
                        nc.scalar.activation(ev[:], ps[:], act_fn.Copy)
                    nc.sync.dma_start(
                        out[mt * 128:(mt + 1) * 128,
                            c * 512:(c + 1) * 512],
                        ev[:])

            for c in range(1, NCH):
                emit_chunk(c)

    nc.compile()
    return nc


def kernel(**inputs) -> np.ndarray:
    _install_profile_hook()
    import ml_dtypes
    bf = ml_dtypes.bfloat16

    x = np.asarray(inputs["x"], dtype=np.float32)
    expert_scores = np.asarray(inputs["expert_scores"], dtype=np.float32)
    W_base = np.asarray(inputs["W_base"], dtype=np.float32)
    b_base = np.asarray(inputs["b_base"], dtype=np.float32)
    gating_W = np.asarray(inputs["gating_W"], dtype=np.float32)
    W_r = np.asarray(inputs["W_r"], dtype=np.float32)
    lora_A = np.asarray(inputs["lora_A"], dtype=np.float32)
    lora_B = np.asarray(inputs["lora_B"], dtype=np.float32)
    module_idx = int(np.asarray(inputs["module_idx"]))
    k = int(np.asarray(inputs["k"]))

    has_bias = bool(np.any(b_base != 0.0))
    key = (k, module_idx, has_bias)
    if key not in _PROGRAM_CACHE:
        _PROGRAM_CACHE[key] = _build_program(k, module_idx, has_bias)
    nc = _PROGRAM_CACHE[key]

    # --- host-side layout prep (transposes/fold/bf16 rounding) ----------
    x_bf = x.astype(bf)                                  # [B, L, D]
    Wt_np = np.ascontiguousarray(
        W_base.T.reshape(KT, 128, NCH, 512).transpose(2, 0, 1, 3)
    ).astype(bf)                                         # [NCH,KT,128,512]
    C = W_r @ gating_W                                   # [NMOD, D] fp32
    ctT_np = np.ascontiguousarray(C.T).astype(bf)        # [D, NMOD]
    A_np = np.ascontiguousarray(
        lora_A.reshape(ER, D).T).astype(bf)              # [D, ER]
    B_np = np.ascontiguousarray(
        lora_B.transpose(0, 2, 1).reshape(ER, D)).astype(bf)  # [ER, D]
    scores_f_np = np.ascontiguousarray(
        expert_scores.T.reshape(1, E * B))               # [1, E*B]
    b_row_np = b_base.reshape(1, D)
    pooledT_np = np.ascontiguousarray(x[:, -1, :].T).astype(bf)  # [D, B]

    in_maps = []
    for c in range(N_CORES):
        msel_np = np.zeros((ER, E, B), dtype=np.float32)
        for p in range(ER):
            msel_np[p, p // R, c] = 1.0
        msel_np = msel_np.reshape(ER, E * B)
        in_maps.append({
            "xT": np.ascontiguousarray(x_bf[c].T),
            "Wt": Wt_np,
            "ctT": ctT_np,
            "pooledT": pooledT_np,
            "scores_f": scores_f_np,
            "A_rhs": A_np,
            "B_cat": B_np,
            "b_row": b_row_np,
            "msel": msel_np,
        })

    from concourse.bass_utils import run_bass_kernel_spmd
    res = run_bass_kernel_spmd(nc, in_maps, core_ids=list(range(N_CORES)))
    return np.stack([res.results[c]["out"] for c in range(N_CORES)], axis=0)


if __name__ == "__main__":
    rng = np.random.default_rng(0)
    demo = {
        "x": (rng.standard_normal((B, L, D)) * 0.02).astype(np.float32),
        "expert_scores": rng.random((B, E), dtype=np.float32),
        "W_base": (rng.standard_normal((D, D)) * 0.02).astype(np.float32),
        "b_base": np.zeros(D, np.float32),
        "gating_W": (rng.standard_normal((D, D)) * 0.02).astype(np.float32),
        "W_r": (rng.standard_normal((NMOD, D)) * 0.02).astype(np.float32),
        "lora_A": (rng.standard_normal((E, R, D)) * 0.02).astype(np.float32),
        "lora_B": (rng.standard_normal((E, D, R)) * 0.02).astype(np.float32),
        "module_idx": 0,
        "k": 2,
    }
    y = kernel(**demo)
    print("out", y.shape, y.dtype, float(np.abs(y).max()))
